# revision 1
# baseline (speedup 1.0000x reference)
"""GPTQ int4 linear (nn_GPTQLinear) on 8 TRN2 NeuronCores — Bass/Tile kernel.

Full problem: x [4, 2048, 4096] fp32, packed int4 weights [4096 x 4096],
groupwise dequant (group size 128), y = x @ W.T + bias -> [4, 2048, 4096].

Sharding: 2-way data-parallel on x rows x 4-way tensor-parallel on
out_features. Per core: M=4096 rows, O=1024 out features, K=4096.

Per-core pipeline (single NEFF, no collectives):
  Phase W: per 128-row o-block, DMA packed bytes; DVE scalar_tensor_tensor
    ((u8 & 15) - zero, (u8 >> 4) - zero, zero broadcast along the group dim
    via step-0 APs) then tensor_tensor multiply by scale -> dequantized fp16
    weight rows [128, 4096] (nibbles interleaved with stride-2 writes);
    DMA-transpose into resident W.T tiles [128p, 32k, 512o] fp16.
  Phase X: per 128-row m-tile, gpsimd cast-DMA fp32->fp16; DMA-transpose to
    X.T [128p, 32k, 128m]; 32 accumulating fp16 matmuls per 512-wide out
    tile + a K=1 matmul adding bias; ACT copies PSUM->SBUF; DMA out fp32.
"""

import sys

if "/opt/trn_rl_repo" not in sys.path:
    sys.path.insert(0, "/opt/trn_rl_repo")

import numpy as np
from contextlib import ExitStack

import concourse.bass as bass
import concourse.tile as tile
import concourse.mybir as mybir
from bass_rust import ScopedClock

F32 = mybir.dt.float32
F16 = mybir.dt.float16
U8 = mybir.dt.uint8
AOP = mybir.AluOpType

# full-problem dims (hardcoded per contract)
B, S, IN_F, OUT_F = 4, 2048, 4096, 4096
GS = 128                       # quant group size == PE contraction tile
N_CORES = 8
DP, TP = 2, 4                  # data-parallel x tensor-parallel split
M_CORE = (B * S) // DP         # 4096
O_CORE = OUT_F // TP           # 1024
K = IN_F                       # 4096


def _split_multi_waits(nc, max_waits=1):
    # The walrus build in this container rejects instructions carrying more
    # than one sync-wait. Move extra waits onto InstNoOp carriers inserted
    # just before the instruction on the same engine (same-engine execution
    # is in-order, so semantics are unchanged).
    n_split = 0
    for fn in nc.m.functions:
        for blk in fn.blocks:
            insts = list(blk.instructions)
            out = []
            for inst in insts:
                si = inst.sync_info
                if (si is not None and len(si.on_wait) > max_waits
                        and inst.engine is not None):
                    w = list(si.on_wait)
                    keep = w[-max_waits:]
                    for j, wx in enumerate(w[:-max_waits]):
                        nop = mybir.InstNoOp(name=f"{inst.name}-w{j}",
                                             ins=[], outs=[])
                        nop.engine = inst.engine
                        nop.sync_info = mybir.SyncInfo(on_wait=[wx],
                                                       on_update=[])
                        nc.register_instruction(nop, overwrite=True)
                        out.append(nop)
                    si.on_wait = keep
                    n_split += 1
                out.append(inst)
            blk.instructions = out
    return n_split


def _patched_drain_and_barrier(self, tick_clock, wait_clock):
    # Same walrus limitation for the Tile tail: the final drain carries one
    # wait per DMA lane. Split the waits across chained drains.
    nc = self.nc
    drain_inst = nc.sync.drain()
    wait_clock.add_sem_waits(drain_inst.ins,
                             ScopedClock({None: tick_clock.global_clock}))
    si = drain_inst.ins.sync_info
    if si is not None:
        w = list(si.on_wait)
        if len(w) > 1:
            si.on_wait = w[:1]
            for extra in w[1:]:
                d2 = nc.sync.drain()
                d2.ins.sync_info = mybir.SyncInfo(on_wait=[extra], on_update=[])
    nc.all_engine_barrier()
    assert self.sems is not None
    popped = nc._tile_sem_poison_stack.pop()
    assert popped is self._sem_poison
    nc.clear_and_free_semaphores(list(self.sems.allocated().values()))
    nc.all_engine_barrier()


tile.TileContext._drain_and_barrier = _patched_drain_and_barrier


def build_nc(M=M_CORE, K_=K, O=O_CORE, GM=6, reps=1):
    # reps>1 repeats the whole computation in one NEFF (benchmarking only)
    P = 128
    NK = K_ // P           # 32 k-tiles == groups
    NM = M // P            # 32 m-tiles
    NOB = O // P           # 8 weight o-blocks
    OT = 512               # out-tile width (one fp32 PSUM bank)
    NOT = O // OT          # 2 out-tiles
    BPG = GS // 2          # 64 packed bytes per group per o-row
    KB = K_ // 2           # 2048 packed bytes per o-row
    NCH = 2                # dequant/matmul k-chunks per K
    NKH = NK // NCH        # 8 groups per k-chunk
    KBH = KB // NCH        # 512 packed bytes per k-chunk
    GM = min(GM, NM)

    nc = bass.Bass("TRN2", target_bir_lowering=False, debug=False,
                   enable_asserts=False)

    xs = nc.dram_tensor("xs", [M, K_], F32, kind="ExternalInput")
    pk = nc.dram_tensor("pk", [O, KB], U8, kind="ExternalInput")
    sc = nc.dram_tensor("sc", [O, NK], F32, kind="ExternalInput")
    zr = nc.dram_tensor("zr", [O, NK], F32, kind="ExternalInput")
    bs = nc.dram_tensor("bs", [O], F32, kind="ExternalInput")
    yo = nc.dram_tensor("yo", [M, O], F32, kind="ExternalOutput")

    with tile.TileContext(nc) as tc, ExitStack() as ctx:
        wt_pool = ctx.enter_context(tc.tile_pool(name="wt", bufs=1))
        wst_pool = ctx.enter_context(tc.tile_pool(name="wst", bufs=3))
        tmp_pool = ctx.enter_context(tc.tile_pool(name="tmp", bufs=3))
        sz_pool = ctx.enter_context(tc.tile_pool(name="sz", bufs=3))
        x_pool = ctx.enter_context(tc.tile_pool(name="x", bufs=3))
        xt_pool = ctx.enter_context(tc.tile_pool(name="xt", bufs=GM + 2))
        y_pool = ctx.enter_context(tc.tile_pool(name="y", bufs=4))
        c_pool = ctx.enter_context(tc.tile_pool(name="c", bufs=1))
        ps_pool = ctx.enter_context(tc.tile_pool(name="ps", bufs=7, space="PSUM"))

        ones = c_pool.tile([1, P], F16, tag="ones")
        nc.vector.memset(ones[:], 1.0)
        bias16 = c_pool.tile([1, O], F16, tag="bias16")
        nc.gpsimd.dma_start(bias16[:], bs[None, :])  # cast f32->f16
        # bias broadcast [P, O] fp32 via two K=1 matmuls (ones.T @ bias16)
        bias_bc = c_pool.tile([P, O], F32, tag="bias_bc")
        for t in range(NOT):
            bp = ps_pool.tile([P, OT], F32, tag="ps", name=f"biasps{t}")
            nc.tensor.matmul(bp[:], ones[:], bias16[:, t * OT:(t + 1) * OT],
                             start=True, stop=True)
            nc.scalar.copy(bias_bc[:, t * OT:(t + 1) * OT], bp[:])

        # one resident W.T tile per out-tile so matmuls only wait on the
        # o-blocks they actually read
        wts = [wt_pool.tile([P, NK, OT], F16, tag=f"wt{t}", name=f"wt{t}")
               for t in range(NOT)]

        # ---- Phase W: dequantize weights ----
        # k-half chunks, ordered so the first out-tile's first k-half is
        # complete as early as possible: [wt0 x kh0], [wt0 x kh1], [wt1 x ...]
        def dequant_half(ob, kh):  # kh = k-chunk index (0..NCH-1)
            r0 = ob * P
            b0 = kh * KBH
            tpk = wst_pool.tile([P, KBH], U8, tag="tpk")
            nc.gpsimd.dma_start(tpk[:], pk[r0:r0 + P, b0:b0 + KBH])
            tz = sz_pool.tile([P, NKH], F32, tag="tz")
            nc.gpsimd.dma_start(tz[:], zr[r0:r0 + P, kh * NKH:(kh + 1) * NKH])
            ts = sz_pool.tile([P, NKH], F32, tag="ts")
            nc.gpsimd.dma_start(ts[:], sc[r0:r0 + P, kh * NKH:(kh + 1) * NKH])

            zap, sap = tz[:], ts[:]
            # broadcast [P, g] -> [P, g, BPG] via step-0 inner dim
            zb = bass.AP(zap.tensor, zap.offset, [zap.ap[0], [1, NKH], [0, BPG]])
            sb = bass.AP(sap.tensor, sap.offset, [sap.ap[0], [1, NKH], [0, BPG]])

            # unpack (bitwise) and dequant (arith) must be separate DVE
            # instructions -- walrus rejects mixed-class op0/op1
            lo_u8 = tmp_pool.tile([P, KBH], U8, tag="nib")
            nc.vector.tensor_scalar(lo_u8[:], tpk[:], 15, None,
                                    op0=AOP.bitwise_and)
            hi_u8 = tmp_pool.tile([P, KBH], U8, tag="nib")
            nc.vector.tensor_scalar(hi_u8[:], tpk[:], 4, None,
                                    op0=AOP.logical_shift_right)

            wblk = wst_pool.tile([P, K_ // NCH], F16, tag="wblk")
            wap = wblk[:]
            # low nibbles are even i_local = 128g + 2b; high nibbles odd
            wev = bass.AP(wap.tensor, wap.offset, [wap.ap[0], [GS, NKH], [2, BPG]])
            wod = bass.AP(wap.tensor, wap.offset + 1,
                          [wap.ap[0], [GS, NKH], [2, BPG]])

            tmp_lo = tmp_pool.tile([P, KBH], F16, tag="tmp")
            tlo = tmp_lo[:].rearrange("p (g b) -> p g b", g=NKH)
            nc.vector.scalar_tensor_tensor(
                tlo, lo_u8[:].rearrange("p (g b) -> p g b", g=NKH), 1.0, zb,
                op0=AOP.mult, op1=AOP.subtract)
            nc.vector.tensor_tensor(wev, tlo, sb, op=AOP.mult)

            tmp_hi = tmp_pool.tile([P, KBH], F16, tag="tmp")
            thi = tmp_hi[:].rearrange("p (g b) -> p g b", g=NKH)
            nc.vector.scalar_tensor_tensor(
                thi, hi_u8[:].rearrange("p (g b) -> p g b", g=NKH), 1.0, zb,
                op0=AOP.mult, op1=AOP.subtract)
            nc.vector.tensor_tensor(wod, thi, sb, op=AOP.mult)

            # transpose [128 o, 2048 i] -> WT[p, kh*16+kloc, o_local]
            t = (ob * P) // OT
            c0 = (ob * P) % OT
            nc.sync.dma_start_transpose(
                wts[t][:, kh * NKH:(kh + 1) * NKH, c0:c0 + P], wblk[:])

        obs_per_wt = OT // P

        # Dequant chunks are emitted lazily, just before the first matmul
        # sweep that reads them. Tile semantics follow program order, and
        # HWDGE dispatch is FIFO per engine -- this keeps xt transposes from
        # queueing behind the whole weight-transpose stream while keeping
        # writes ahead of reads.
        emitted = set()

        def need_chunk(t, kh):
            if (t, kh) in emitted:
                return
            emitted.add((t, kh))
            for obl in range(obs_per_wt):
                dequant_half(t * obs_per_wt + obl, kh)

        # ---- Phase X + matmul ----
        # Sweep all m-tiles per out-tile (X is cast+transposed once per ot --
        # cheap) so the PE only ever waits on the *first* W.T tile. For the
        # first GM m-tiles, additionally sweep k-halves across m-tiles so the
        # PE can start as soon as the first k-half of wts[0] is transposed.
        def load_xt(mt, delay_ms=None):
            m0 = mt * P
            with tc.tile_wait_until(delay_ms, enable=delay_ms is not None):
                x16 = x_pool.tile([P, K_], F16, tag="x16")
                nc.gpsimd.dma_start(x16[:], xs[m0:m0 + P, :])  # cast f32->f16
                xt = xt_pool.tile([P, NK, P], F16, tag="xt")
                # All transposes stay on the sync (SP) HWDGE queue -- issuing
                # DmaTransposeAnt from two queues races the xbar (observed
                # data corruption on HW).
                nc.sync.dma_start_transpose(xt[:], x16[:])
            return xt

        def finish_tile(mt, ot, ps):
            o0 = ot * OT
            yst = y_pool.tile([P, OT], F32, tag="yst")
            nc.vector.tensor_tensor(yst[:], ps[:], bias_bc[:, o0:o0 + OT],
                                    op=AOP.add)
            nc.scalar.dma_start(yo[mt * P:mt * P + P, o0:o0 + OT], yst[:])

        def phase_x(rep):
            # Emit the first weight chunk, then group-0's X transposes, then
            # the rest of the dequant. HWDGE dispatch is FIFO per engine, so
            # this lets xt0 dispatch after only 4 weight transposes instead
            # of all 16 (the last of which is DVE-paced ~90us in). The DVE
            # stream stays uninterrupted dequant throughout.
            need_chunk(0, 0)
            first = True
            for g0 in range(0, NM, GM):
                mts = list(range(g0, min(g0 + GM, NM)))
                xts = {mt: load_xt(mt) for mt in mts}
                if first:
                    first = False
                    for t in range(NOT):
                        for kh in range(NCH):
                            need_chunk(t, kh)
                for ot in range(NOT):
                    # sweep k-halves across the group's m-tiles so early
                    # matmuls only wait on the first k-half of wts[ot]
                    pss = {mt: ps_pool.tile([P, OT], F32, tag="ps",
                                            name=f"ps{rep}_{g0}_{ot}_{mt}")
                           for mt in mts}
                    for kh in range(NCH):
                        need_chunk(ot, kh)
                        for mt in mts:
                            for k in range(kh * NKH, (kh + 1) * NKH):
                                nc.tensor.matmul(pss[mt][:], xts[mt][:, k, :],
                                                 wts[ot][:, k, :],
                                                 start=(k == 0),
                                                 stop=(k == NK - 1))
                    for mt in mts:
                        finish_tile(mt, ot, pss[mt])

        phase_x(0)
        for rep in range(1, reps):
            emitted.clear()
            phase_x(rep)

    _split_multi_waits(nc)
    return nc


_CACHED_NC = None


def _get_nc():
    global _CACHED_NC
    if _CACHED_NC is None:
        _CACHED_NC = build_nc()
    return _CACHED_NC


def make_in_maps(x, scale, zero, bias, packed_weight):
    x2 = np.ascontiguousarray(np.asarray(x, dtype=np.float32).reshape(B * S, IN_F))
    pk_all = np.asarray(packed_weight, dtype=np.int32)
    pk8 = np.ascontiguousarray(pk_all.view(np.uint8).reshape(OUT_F, IN_F // 2))
    scale = np.asarray(scale, dtype=np.float32)
    zero = np.asarray(zero, dtype=np.float32)
    bias = np.asarray(bias, dtype=np.float32)

    in_maps = []
    for c in range(N_CORES):
        mb, ob = c // TP, c % TP
        in_maps.append({
            "xs": np.ascontiguousarray(x2[mb * M_CORE:(mb + 1) * M_CORE]),
            "pk": np.ascontiguousarray(pk8[ob * O_CORE:(ob + 1) * O_CORE]),
            "sc": np.ascontiguousarray(scale[ob * O_CORE:(ob + 1) * O_CORE]),
            "zr": np.ascontiguousarray(zero[ob * O_CORE:(ob + 1) * O_CORE]),
            "bs": np.ascontiguousarray(bias[ob * O_CORE:(ob + 1) * O_CORE]),
        })
    return in_maps


def assemble(results):
    y = np.empty((B * S, OUT_F), dtype=np.float32)
    for c in range(N_CORES):
        mb, ob = c // TP, c % TP
        y[mb * M_CORE:(mb + 1) * M_CORE,
          ob * O_CORE:(ob + 1) * O_CORE] = results[c]["yo"]
    return y.reshape(B, S, OUT_F)


def kernel(x, scale, zero, bias, packed_weight, trace=False):
    from concourse.bass_utils import run_bass_kernel_spmd
    nc = _get_nc()
    in_maps = make_in_maps(x, scale, zero, bias, packed_weight)
    res = run_bass_kernel_spmd(nc, in_maps, core_ids=list(range(N_CORES)),
                               trace=trace)
    out = assemble(res.results)
    if trace:
        kernel.last_result = res
    return out



# revision 2
# speedup vs baseline: 2.1206x; 2.1206x over previous
"""GPTQ int4 linear (nn_GPTQLinear) on 8 TRN2 NeuronCores — Bass/Tile kernel.

Full problem: x [4, 2048, 4096] fp32, packed int4 weights [4096 x 4096],
groupwise dequant (group size 128), y = x @ W.T + bias -> [4, 2048, 4096].

Sharding: 2-way data-parallel on x rows x 4-way tensor-parallel on
out_features. Per core: M=4096 rows, O=1024 out features, K=4096.

v2 design (PE-roofline): x is pre-cast to fp16 and pre-transposed on the
HOST into K-major tiled layout [mw, k, kt, m] so the device reads X.T
windows [128k, 32kt, 512m] straight from HBM (one 4 MB contiguous DMA per
window on the gpsimd/SWDGE queue) — no on-device cast pass and no X
SBUF->SBUF transposes (the v1 x path cost ~96 MB of SBUF-fabric traffic
per rep). Weight path unchanged: per 128-row o-block, DVE unpack+dequant
to fp16, sync-queue DMA-transpose into resident W.T tiles [128, 32k, 512o].
Matmuls: per (window, out-tile, 128-m chunk) one PSUM bank accumulates 32
k-matmuls (kh-split so the PE can start after half a weight tile) plus a
K=1 ones-x-bias matmul; ACT copies PSUM->SBUF and DMAs out on the scalar
queue. pk/scale/zero loads ride the scalar queue too.
"""

import sys

if "/opt/trn_rl_repo" not in sys.path:
    sys.path.insert(0, "/opt/trn_rl_repo")

import numpy as np
from contextlib import ExitStack

import concourse.bass as bass
import concourse.tile as tile
import concourse.mybir as mybir
from bass_rust import ScopedClock

F32 = mybir.dt.float32
F16 = mybir.dt.float16
U8 = mybir.dt.uint8
AOP = mybir.AluOpType

# full-problem dims (hardcoded per contract)
B, S, IN_F, OUT_F = 4, 2048, 4096, 4096
GS = 128                       # quant group size == PE contraction tile
N_CORES = 8
DP, TP = 2, 4                  # data-parallel x tensor-parallel split
M_CORE = (B * S) // DP         # 4096
O_CORE = OUT_F // TP           # 1024
K = IN_F                       # 4096


def _split_multi_waits(nc, max_waits=1):
    # The walrus build in this container rejects instructions carrying more
    # than one sync-wait. Move extra waits onto InstNoOp carriers inserted
    # just before the instruction on the same engine (same-engine execution
    # is in-order, so semantics are unchanged).
    n_split = 0
    for fn in nc.m.functions:
        for blk in fn.blocks:
            insts = list(blk.instructions)
            out = []
            for inst in insts:
                si = inst.sync_info
                if (si is not None and len(si.on_wait) > max_waits
                        and inst.engine is not None):
                    w = list(si.on_wait)
                    keep = w[-max_waits:]
                    for j, wx in enumerate(w[:-max_waits]):
                        nop = mybir.InstNoOp(name=f"{inst.name}-w{j}",
                                             ins=[], outs=[])
                        nop.engine = inst.engine
                        nop.sync_info = mybir.SyncInfo(on_wait=[wx],
                                                       on_update=[])
                        nc.register_instruction(nop, overwrite=True)
                        out.append(nop)
                    si.on_wait = keep
                    n_split += 1
                out.append(inst)
            blk.instructions = out
    return n_split


def _patched_drain_and_barrier(self, tick_clock, wait_clock):
    # Same walrus limitation for the Tile tail: the final drain carries one
    # wait per DMA lane. Split the waits across chained drains.
    nc = self.nc
    drain_inst = nc.sync.drain()
    wait_clock.add_sem_waits(drain_inst.ins,
                             ScopedClock({None: tick_clock.global_clock}))
    si = drain_inst.ins.sync_info
    if si is not None:
        w = list(si.on_wait)
        if len(w) > 1:
            si.on_wait = w[:1]
            for extra in w[1:]:
                d2 = nc.sync.drain()
                d2.ins.sync_info = mybir.SyncInfo(on_wait=[extra], on_update=[])
    nc.all_engine_barrier()
    assert self.sems is not None
    popped = nc._tile_sem_poison_stack.pop()
    assert popped is self._sem_poison
    nc.clear_and_free_semaphores(list(self.sems.allocated().values()))
    nc.all_engine_barrier()


tile.TileContext._drain_and_barrier = _patched_drain_and_barrier


def build_nc(M=M_CORE, K_=K, O=O_CORE, reps=1):
    # reps>1 repeats the whole computation in one NEFF (benchmarking only)
    P = 128
    NK = K_ // P           # 32 k-tiles == groups
    MW = 512               # m-window width
    NMW = M // MW          # 8 windows
    NMC = MW // P          # 4 m-chunks per window
    NOB = O // P           # 8 weight o-blocks
    OT = 512               # out-tile width (one fp32 PSUM bank)
    NOT = O // OT          # 2 out-tiles
    BPG = GS // 2          # 64 packed bytes per group per o-row
    KB = K_ // 2           # 2048 packed bytes per o-row
    NCH = 2                # dequant/matmul k-chunks per K
    NKH = NK // NCH        # 16 groups per k-chunk
    KBH = KB // NCH        # 1024 packed bytes per k-chunk

    nc = bass.Bass("TRN2", target_bir_lowering=False, debug=False,
                   enable_asserts=False)

    xt_d = nc.dram_tensor("xt", [NMW, P, NK, MW], F16, kind="ExternalInput")
    pk = nc.dram_tensor("pk", [O, KB], U8, kind="ExternalInput")
    sc = nc.dram_tensor("sc", [O, NK], F32, kind="ExternalInput")
    zr = nc.dram_tensor("zr", [O, NK], F32, kind="ExternalInput")
    bs = nc.dram_tensor("bs", [O], F32, kind="ExternalInput")
    yo = nc.dram_tensor("yo", [M, O], F32, kind="ExternalOutput")

    with tile.TileContext(nc) as tc, ExitStack() as ctx:
        wt_pool = ctx.enter_context(tc.tile_pool(name="wt", bufs=1))
        wst_pool = ctx.enter_context(tc.tile_pool(name="wst", bufs=3))
        tmp_pool = ctx.enter_context(tc.tile_pool(name="tmp", bufs=3))
        sz_pool = ctx.enter_context(tc.tile_pool(name="sz", bufs=3))
        xw_pool = ctx.enter_context(tc.tile_pool(name="xw", bufs=2))
        y_pool = ctx.enter_context(tc.tile_pool(name="y", bufs=4))
        c_pool = ctx.enter_context(tc.tile_pool(name="c", bufs=1))
        ps_pool = ctx.enter_context(tc.tile_pool(name="ps", bufs=8, space="PSUM"))

        ones = c_pool.tile([1, P], F16, tag="ones")
        nc.vector.memset(ones[:], 1.0)
        bias16 = c_pool.tile([1, O], F16, tag="bias16")
        nc.gpsimd.dma_start(bias16[:], bs[None, :])  # cast f32->f16

        # one resident W.T tile per out-tile so matmuls only wait on the
        # o-blocks they actually read
        wts = [wt_pool.tile([P, NK, OT], F16, tag=f"wt{t}", name=f"wt{t}")
               for t in range(NOT)]

        # ---- Phase W: dequantize weights ----
        def dequant_half(ob, kh):  # kh = k-chunk index (0..NCH-1)
            r0 = ob * P
            b0 = kh * KBH
            tpk = wst_pool.tile([P, KBH], U8, tag="tpk")
            nc.scalar.dma_start(tpk[:], pk[r0:r0 + P, b0:b0 + KBH])
            tz = sz_pool.tile([P, NKH], F32, tag="tz")
            nc.scalar.dma_start(tz[:], zr[r0:r0 + P, kh * NKH:(kh + 1) * NKH])
            ts = sz_pool.tile([P, NKH], F32, tag="ts")
            nc.scalar.dma_start(ts[:], sc[r0:r0 + P, kh * NKH:(kh + 1) * NKH])

            zap, sap = tz[:], ts[:]
            # broadcast [P, g] -> [P, g, BPG] via step-0 inner dim
            zb = bass.AP(zap.tensor, zap.offset, [zap.ap[0], [1, NKH], [0, BPG]])
            sb = bass.AP(sap.tensor, sap.offset, [sap.ap[0], [1, NKH], [0, BPG]])

            # unpack (bitwise) and dequant (arith) must be separate DVE
            # instructions -- walrus rejects mixed-class op0/op1
            lo_u8 = tmp_pool.tile([P, KBH], U8, tag="nib")
            nc.vector.tensor_scalar(lo_u8[:], tpk[:], 15, None,
                                    op0=AOP.bitwise_and)
            hi_u8 = tmp_pool.tile([P, KBH], U8, tag="nib")
            nc.vector.tensor_scalar(hi_u8[:], tpk[:], 4, None,
                                    op0=AOP.logical_shift_right)

            wblk = wst_pool.tile([P, K_ // NCH], F16, tag="wblk")
            wap = wblk[:]
            # low nibbles are even i_local = 128g + 2b; high nibbles odd
            wev = bass.AP(wap.tensor, wap.offset, [wap.ap[0], [GS, NKH], [2, BPG]])
            wod = bass.AP(wap.tensor, wap.offset + 1,
                          [wap.ap[0], [GS, NKH], [2, BPG]])

            tmp_lo = tmp_pool.tile([P, KBH], F16, tag="tmp")
            tlo = tmp_lo[:].rearrange("p (g b) -> p g b", g=NKH)
            nc.vector.scalar_tensor_tensor(
                tlo, lo_u8[:].rearrange("p (g b) -> p g b", g=NKH), 1.0, zb,
                op0=AOP.mult, op1=AOP.subtract)
            nc.vector.tensor_tensor(wev, tlo, sb, op=AOP.mult)

            tmp_hi = tmp_pool.tile([P, KBH], F16, tag="tmp")
            thi = tmp_hi[:].rearrange("p (g b) -> p g b", g=NKH)
            nc.vector.scalar_tensor_tensor(
                thi, hi_u8[:].rearrange("p (g b) -> p g b", g=NKH), 1.0, zb,
                op0=AOP.mult, op1=AOP.subtract)
            nc.vector.tensor_tensor(wod, thi, sb, op=AOP.mult)

            # transpose [128 o, 2048 i] -> WT[p, kh*16+kloc, o_local]
            t = (ob * P) // OT
            c0 = (ob * P) % OT
            nc.sync.dma_start_transpose(
                wts[t][:, kh * NKH:(kh + 1) * NKH, c0:c0 + P], wblk[:])

        obs_per_wt = OT // P

        # Dequant chunks are emitted lazily, just before the first matmul
        # sweep that reads them (keeps DVE ahead of the PE with writes
        # ahead of reads).
        emitted = set()

        def need_chunk(t, kh):
            if (t, kh) in emitted:
                return
            emitted.add((t, kh))
            for obl in range(obs_per_wt):
                dequant_half(t * obs_per_wt + obl, kh)

        # ---- Phase X + matmul ----
        def phase_x(rep):
            need_chunk(0, 0)
            for mw in range(NMW):
                xw = xw_pool.tile([P, NK, MW], F16, tag="xw",
                                  name=f"xw{rep}_{mw}")
                nc.gpsimd.dma_start(xw[:], xt_d[mw])
                for ot in range(NOT):
                    pss = [ps_pool.tile([P, OT], F32, tag="ps",
                                        name=f"ps{rep}_{mw}_{ot}_{mc}")
                           for mc in range(NMC)]
                    # kh-split sweep so the first groups only wait on the
                    # first half of wts[ot]
                    for kh in range(NCH):
                        need_chunk(ot, kh)
                        for mc in range(NMC):
                            for k in range(kh * NKH, (kh + 1) * NKH):
                                nc.tensor.matmul(
                                    pss[mc][:],
                                    xw[:, k, mc * P:(mc + 1) * P],
                                    wts[ot][:, k, :],
                                    start=(k == 0), stop=False)
                    o0 = ot * OT
                    for mc in range(NMC):
                        nc.tensor.matmul(pss[mc][:], ones[:],
                                         bias16[:, o0:o0 + OT],
                                         start=False, stop=True,
                                         skip_group_check=True)
                        yst = y_pool.tile([P, OT], F32, tag="yst",
                                          name=f"y{rep}_{mw}_{ot}_{mc}")
                        nc.scalar.copy(yst[:], pss[mc][:])
                        m0 = mw * MW + mc * P
                        nc.scalar.dma_start(yo[m0:m0 + P, o0:o0 + OT], yst[:])

        phase_x(0)
        for rep in range(1, reps):
            emitted.clear()
            phase_x(rep)

    _split_multi_waits(nc)
    return nc


_CACHED_NC = None


def _get_nc():
    global _CACHED_NC
    if _CACHED_NC is None:
        _CACHED_NC = build_nc()
    return _CACHED_NC


def make_in_maps(x, scale, zero, bias, packed_weight):
    x2 = np.asarray(x, dtype=np.float32).reshape(B * S, IN_F)
    pk_all = np.asarray(packed_weight, dtype=np.int32)
    pk8 = np.ascontiguousarray(pk_all.view(np.uint8).reshape(OUT_F, IN_F // 2))
    scale = np.asarray(scale, dtype=np.float32)
    zero = np.asarray(zero, dtype=np.float32)
    bias = np.asarray(bias, dtype=np.float32)

    # host-side pre-cast + pre-transpose of x into K-major tiled layout:
    # xt[mw, k, kt, m] = x_half[mw*512 + m, kt*128 + k]  (fp16)
    xt_halves = []
    for mb in range(DP):
        xh = x2[mb * M_CORE:(mb + 1) * M_CORE].astype(np.float16)
        a = xh.reshape(M_CORE // 512, 512, IN_F // 128, 128)
        xt_halves.append(np.ascontiguousarray(a.transpose(0, 3, 2, 1)))

    in_maps = []
    for c in range(N_CORES):
        mb, ob = c // TP, c % TP
        in_maps.append({
            "xt": xt_halves[mb],
            "pk": np.ascontiguousarray(pk8[ob * O_CORE:(ob + 1) * O_CORE]),
            "sc": np.ascontiguousarray(scale[ob * O_CORE:(ob + 1) * O_CORE]),
            "zr": np.ascontiguousarray(zero[ob * O_CORE:(ob + 1) * O_CORE]),
            "bs": np.ascontiguousarray(bias[ob * O_CORE:(ob + 1) * O_CORE]),
        })
    return in_maps


def assemble(results):
    y = np.empty((B * S, OUT_F), dtype=np.float32)
    for c in range(N_CORES):
        mb, ob = c // TP, c % TP
        y[mb * M_CORE:(mb + 1) * M_CORE,
          ob * O_CORE:(ob + 1) * O_CORE] = results[c]["yo"]
    return y.reshape(B, S, OUT_F)


def kernel(x, scale, zero, bias, packed_weight, trace=False):
    from concourse.bass_utils import run_bass_kernel_spmd
    nc = _get_nc()
    in_maps = make_in_maps(x, scale, zero, bias, packed_weight)
    res = run_bass_kernel_spmd(nc, in_maps, core_ids=list(range(N_CORES)),
                               trace=trace)
    out = assemble(res.results)
    if trace:
        kernel.last_result = res
    return out
